# revision 21
# baseline (speedup 1.0000x reference)
"""Causal self-attention on 8 NeuronCores.

Sharding: data-parallel over batch (B=2) x tensor-parallel over heads
(16 heads -> 4 groups of 4), Megatron-style. Core c handles batch c//4,
head-group c%4. Each core computes its QKV projection slice, 4 heads of
causal attention, and a partial output projection; the host sums the 4
partials per batch element.

Single pipelined loop over 512-query chunks (S=2048, D=1024, HD=64,
4 local heads, fp16 operands / fp32 accumulation):
  x is transposed + fp16-cast on the host, so no on-device transposes.
  Attention tiles are processed in 2-tile units sharing one PSUM score
  tile and one ACT exp call; the diagonal band is causally trimmed
  (widths 512/384/256/128) with 128-wide masks on Pool. The qkT/v
  projections of chunk j+1 and the output projection of chunk j-1 are
  interleaved as PE filler between attention heads of chunk j, keeping
  every engine busy. Softmax denominators ride a fused ones-row in the
  PV matmul; normalization = DVE reciprocal + Pool partition-broadcast
  + DVE multiply.
"""

import sys

sys.path.insert(0, "/opt/trn_rl_repo")

import numpy as np

import concourse.bass as bass
import concourse.bacc as bacc
import concourse.mybir as mybir
from concourse.bass import ts, ds
from concourse.tile import TileContext

B, S, D, H = 2, 2048, 1024, 16
HD = D // H  # 64
NH = 4  # heads per core
P = 128
DT = D // P  # 8 d-tiles
ST = S // P  # 16 s-tiles
NCHUNK = 4  # sq chunks of 512
CH = 512
F32 = mybir.dt.float32
F16 = mybir.dt.float16
LAU = 2  # score->PV unit lookahead

_nc_cache = None


def build_nc():
    nc = bacc.Bacc("TRN2", target_bir_lowering=False, debug=False, num_devices=8)

    xT_d = nc.dram_tensor("xT", [D, S], F16, kind="ExternalInput")
    wqk_d = nc.dram_tensor("wqk", [D, 2 * NH * HD], F16, kind="ExternalInput")
    wv_d = nc.dram_tensor("wv", [D, NH * HD], F16, kind="ExternalInput")
    wp_d = nc.dram_tensor("wp", [NH * HD, D], F16, kind="ExternalInput")
    bqk_d = nc.dram_tensor("bqk", [2 * NH * HD], F32, kind="ExternalInput")
    bv_d = nc.dram_tensor("bv", [NH * (HD + 1)], F32, kind="ExternalInput")
    bp_d = nc.dram_tensor("bp", [D], F32, kind="ExternalInput")
    out_d = nc.dram_tensor("out", [S, D], F32, kind="ExternalOutput")

    with TileContext(nc) as tc:
        import contextlib

        stack = contextlib.ExitStack()
        with stack:
            consts = stack.enter_context(tc.tile_pool(name="consts", bufs=1))
            xt_pool = stack.enter_context(tc.tile_pool(name="xt", bufs=2))
            yt_pool = stack.enter_context(tc.tile_pool(name="yt", bufs=4))
            pt_pool = stack.enter_context(tc.tile_pool(name="pt", bufs=6))
            ot_pool = stack.enter_context(tc.tile_pool(name="ot", bufs=3))
            rec_pool = stack.enter_context(tc.tile_pool(name="rec", bufs=3))
            mm_ps = stack.enter_context(tc.tile_pool(name="mm", bufs=2, space="PSUM"))
            sc_ps = stack.enter_context(tc.tile_pool(name="sc", bufs=2, space="PSUM"))
            pv_ps = stack.enter_context(tc.tile_pool(name="pv", bufs=2, space="PSUM"))

            # ---- weights / constants ----
            wqk_sb = consts.tile([P, DT, 2 * NH * HD], F16)
            wv_sb = consts.tile([P, DT, NH * HD], F16)
            wp_sb = consts.tile([P, 2, D], F16)
            bqk_sb = consts.tile([P, 4], F32)
            bv_sb = consts.tile([1, NH * (HD + 1)], F32)
            bv_rep = consts.tile([P, NH * (HD + 1)], F32)
            bp_sb = consts.tile([1, D], F32)
            bp_rep = consts.tile([P, D], F32)

            # persistent activations
            qkT_sb = consts.tile([P, 4, S], F16)  # q: tiles 0-1, k: tiles 2-3
            v_sb = consts.tile([P, ST, NH * (HD + 1)], F16)
            v4 = v_sb.rearrange("p st (h c) -> p st h c", c=HD + 1)
            bv4 = bv_rep.rearrange("p (h c) -> p h c", c=HD + 1)

            xT_ap = xT_d[:].rearrange("(dt p) s -> p dt s", p=P)
            wqk_ap = wqk_d[:].rearrange("(do p) f -> p do f", p=P)

            # ---- prologue DMAs, split + ordered for fast PE start ----
            nc.sync.dma_start(wqk_sb[:, :, :P], wqk_ap[:, :, :P])
            xts = {0: xt_pool.tile([P, DT, CH], F16, name="xt")}
            nc.sync.dma_start(xts[0][:, :4, :], xT_ap[:, :4, ds(0, CH)])
            nc.sync.dma_start(xts[0][:, 4:, :], xT_ap[:, 4:, ds(0, CH)])
            nc.sync.dma_start(bqk_sb, bqk_d[:].rearrange("(t p) -> p t", p=P))
            nc.sync.dma_start(
                wqk_sb[:, :, ts(2, P)], wqk_ap[:, :, ts(2, P)]
            )
            nc.sync.dma_start(wqk_sb[:, :, ts(1, P)], wqk_ap[:, :, ts(1, P)])
            nc.sync.dma_start(wqk_sb[:, :, ts(3, P)], wqk_ap[:, :, ts(3, P)])
            nc.sync.dma_start(wv_sb, wv_d[:].rearrange("(do p) f -> p do f", p=P))
            nc.sync.dma_start(bv_sb, bv_d[:].unsqueeze(0))
            nc.gpsimd.partition_broadcast(bv_rep, bv_sb)
            nc.sync.dma_start(wp_sb, wp_d[:].rearrange("(i p) f -> p i f", p=P))
            nc.sync.dma_start(bp_sb, bp_d[:].unsqueeze(0))
            nc.gpsimd.partition_broadcast(bp_rep, bp_sb)
            nc.gpsimd.memset(v4[:, :, :, HD : HD + 1], 1.0)

            # ---- filler unit emitters ----
            def emit_qk_ft(j, ft):
                xt = xts[j]
                pp = mm_ps.tile([P, CH], F32, name="pp", tag="mm")
                for do in range(DT):
                    nc.tensor.matmul(
                        pp,
                        wqk_sb[:, do, ts(ft, P)],
                        xt[:, do, :],
                        start=(do == 0),
                        stop=(do == DT - 1),
                    )
                nc.vector.tensor_scalar_add(
                    qkT_sb[:, ft, ds(j * CH, CH)], pp, bqk_sb[:, ft : ft + 1]
                )

            def emit_v_st(j, st4):
                xt = xts[j]
                st = 4 * j + st4
                pv = mm_ps.tile([P, CH], F32, name="pv", tag="mm")
                for do in range(DT):
                    nc.tensor.matmul(
                        pv[:, : NH * HD],
                        xt[:, do, ts(st4, P)],
                        wv_sb[:, do, :],
                        start=(do == 0),
                        stop=(do == DT - 1),
                    )
                nc.vector.tensor_tensor(
                    v4[:, st, :, :HD],
                    pv[:, : NH * HD].rearrange("p (h c) -> p h c", c=HD),
                    bv4[:, :, :HD],
                    mybir.AluOpType.add,
                )

            def emit_outproj_st(j, yt, st4, alt=False):
                o_t = ot_pool.tile([P, D], F32, name="o_t")
                for c in range(2):
                    if alt and c == 1:
                        op = sc_ps.tile([P, 2 * CH], F32, name="op2", tag="sc")[
                            :, :CH
                        ]
                    else:
                        op = mm_ps.tile([P, CH], F32, name="op", tag="mm")
                    nc.tensor.matmul(
                        op,
                        yt[:, 0, ts(st4, P)],
                        wp_sb[:, 0, ds(c * CH, CH)],
                        start=True,
                        stop=False,
                    )
                    nc.tensor.matmul(
                        op,
                        yt[:, 1, ts(st4, P)],
                        wp_sb[:, 1, ds(c * CH, CH)],
                        start=False,
                        stop=True,
                    )
                    nc.vector.tensor_tensor(
                        o_t[:, ds(c * CH, CH)],
                        op,
                        bp_rep[:, ds(c * CH, CH)],
                        mybir.AluOpType.add,
                    )
                nc.sync.dma_start(out_d[ts(4 * j + st4, P), :], o_t)

            def proj_units(j):
                return [lambda ft=ft: emit_qk_ft(j, ft) for ft in (0, 2, 1, 3)] + [
                    lambda s=s: emit_v_st(j, s) for s in range(4)
                ]

            # prologue: chunk-0 projections
            for u in proj_units(0):
                u()

            # ---- main loop ----
            # outproj of chunk jp is deferred into chunk DEFER[jp] as PE
            # filler (chunk 3 has no projection filler and is ACT-paced)
            DEFER = {0: 3, 1: 3, 2: 3}
            yts = {}
            for j in range(NCHUNK):
                filler = []
                if j + 1 < NCHUNK:
                    xts[j + 1] = xt_pool.tile([P, DT, CH], F16, name="xt")
                    nc.sync.dma_start(
                        xts[j + 1], xT_ap[:, :, ds((j + 1) * CH, CH)]
                    )
                for jp, jd in DEFER.items():
                    if jd == j:
                        yp = yts.pop(jp)
                        filler += [
                            lambda s=s, yp=yp, jp=jp: emit_outproj_st(jp, yp, s)
                            for s in range(4)
                        ]
                if j + 1 < NCHUNK:
                    filler += proj_units(j + 1)

                # fractional filler pacing across this chunk's attention units;
                # the last chunk reserves units to cover its tail norm chain
                tail_units = []
                if j == NCHUNK - 1:
                    tail_units = filler[-3:]
                    filler = filler[:-3]
                total_units = NH * (2 + 2 * j)
                frac = len(filler) / total_units
                acc = -2.0 * frac if j == 0 else 0.0  # chunk 0 waits on xt DMA

                def maybe_filler():
                    nonlocal acc
                    acc += frac
                    while acc >= 1.0 and filler:
                        filler.pop(0)()
                        acc -= 1.0

                yt = yt_pool.tile([P, 2, CH], F16, name="yt")
                yts[j] = yt
                pending = []  # (h, pvp, rec) queue awaiting broadcast+normalize

                def flush_norm():
                    if not pending:
                        return
                    hN, pvpN, recN = pending.pop(0)
                    recb = rec_pool.tile([HD, CH], F32, name="recb")
                    nc.gpsimd.partition_broadcast(recb, recN)
                    nc.vector.tensor_tensor(
                        yt[(hN % 2) * HD : (hN % 2) * HD + HD, hN // 2, :],
                        pvpN[0:HD, :],
                        recb,
                        mybir.AluOpType.mult,
                    )

                # units of <=2 tiles: (tile, width, q-offset, psum-offset)
                units = [
                    [(4 * j, CH, 0, 0), (4 * j + 1, 384, P, CH)],
                    [(4 * j + 2, 256, 2 * P, 0), (4 * j + 3, P, 3 * P, 256)],
                ] + [
                    [(d, CH, 0, 0), (d + 1, CH, 0, CH)]
                    for d in range(0, 4 * j, 2)
                ]
                nu = len(units)
                pvps = {}
                pts = {}

                def emit_scores(h, ui):
                    po = (h % 2) * HD
                    unit = units[ui]
                    sc = sc_ps.tile([P, 2 * CH], F32, name="sc", tag="sc")
                    for t, w, qoff, off in unit:
                        nc.tensor.matmul(
                            sc[:, ds(off, w)],
                            qkT_sb[po : po + HD, 2 + h // 2, ts(t, P)],
                            qkT_sb[po : po + HD, h // 2, ds(j * CH + qoff, w)],
                            start=True,
                            stop=True,
                        )
                    cov = unit[-1][3] + unit[-1][1]
                    pt = pt_pool.tile([P, 2 * CH], F16, name="pt")
                    nc.scalar.activation(
                        pt[:, :cov], sc[:, :cov],
                        mybir.ActivationFunctionType.Exp, scale=0.125,
                    )
                    if ui < 2:
                        for t, w, qoff, off in unit:
                            # mask upper triangle of the diagonal block
                            nc.gpsimd.affine_select(
                                out=pt[:, ds(off, P)],
                                in_=pt[:, ds(off, P)],
                                compare_op=mybir.AluOpType.is_ge,
                                fill=0.0,
                                base=0,
                                pattern=[[1, P]],
                                channel_multiplier=-1,
                            )
                    pts[(h, ui)] = pt

                def emit_pv(h, ui):
                    if ui == 0:
                        pvps[h] = pv_ps.tile([P, CH], F32, name="pvp", tag="pv")
                    pvp = pvps[h]
                    unit = units[ui]
                    pt = pts.pop((h, ui))
                    for k, (t, w, qoff, off) in enumerate(unit):
                        nc.tensor.matmul(
                            pvp[: HD + 1, ds(qoff, w)],
                            v_sb[:, t, ds(h * (HD + 1), HD + 1)],
                            pt[:, ds(off, w)],
                            start=(ui == 0 and k == 0),
                            stop=(ui == nu - 1 and k == len(unit) - 1),
                            skip_group_check=True,
                        )
                    if ui == nu - 1:
                        rec = rec_pool.tile([1, CH], F32, name="rec")
                        nc.vector.reciprocal(rec, pvp[HD : HD + 1, :])
                        pending.append((h, pvp, rec))

                stream = [(h, ui) for h in range(NH) for ui in range(nu)]
                for idx in range(len(stream) + LAU):
                    if idx < len(stream):
                        emit_scores(*stream[idx])
                    if idx >= LAU:
                        h, ui = stream[idx - LAU]
                        emit_pv(h, ui)
                        if ui == min(1, nu - 1) and h > 0:
                            flush_norm()  # normalize head h-1
                    maybe_filler()

                # tail: remaining filler covers the last norm chain latency
                while filler:
                    filler.pop(0)()
                for u in tail_units:
                    u()
                while pending:
                    flush_norm()

            # final chunk's output projection (alternate PSUM pools: the
            # attention pools are free by now, so rotation never stalls)
            yp = yts.pop(NCHUNK - 1)
            for st4 in range(4):
                emit_outproj_st(NCHUNK - 1, yp, st4, alt=True)

    nc.compile()
    return nc


def make_in_maps(x, W_attn, b_attn, W_proj, b_proj):
    x = np.asarray(x, dtype=np.float32)
    W_attn = np.asarray(W_attn, dtype=np.float32)
    b_attn = np.asarray(b_attn, dtype=np.float32)
    W_proj = np.asarray(W_proj, dtype=np.float32)
    b_proj = np.asarray(b_proj, dtype=np.float32)
    GF = NH * HD  # 256 features per group
    in_maps = []
    for c in range(8):
        b, g = divmod(c, 4)
        sl = slice(g * GF, (g + 1) * GF)
        wqk = np.concatenate(
            [W_attn[:, sl], W_attn[:, D + g * GF : D + (g + 1) * GF]], axis=1
        )
        bqk = np.concatenate([b_attn[sl], b_attn[D + g * GF : D + (g + 1) * GF]])
        wv = W_attn[:, 2 * D + g * GF : 2 * D + (g + 1) * GF]
        bv_flat = b_attn[2 * D + g * GF : 2 * D + (g + 1) * GF]
        bv = np.zeros(NH * (HD + 1), dtype=np.float32)
        for h in range(NH):
            bv[h * (HD + 1) : h * (HD + 1) + HD] = bv_flat[h * HD : (h + 1) * HD]
        in_maps.append(
            {
                "xT": np.ascontiguousarray(x[b].T.astype(np.float16)),
                "wqk": np.ascontiguousarray(wqk.astype(np.float16)),
                "wv": np.ascontiguousarray(wv.astype(np.float16)),
                "wp": np.ascontiguousarray(W_proj[sl, :].astype(np.float16)),
                "bqk": np.ascontiguousarray(bqk),
                "bv": bv,
                "bp": (b_proj if g == 0 else np.zeros_like(b_proj)).copy(),
            }
        )
    return in_maps


def kernel(x, W_attn, b_attn, W_proj, b_proj):
    global _nc_cache
    from concourse.bass_utils import run_bass_kernel_spmd

    if _nc_cache is None:
        _nc_cache = build_nc()
    nc = _nc_cache
    in_maps = make_in_maps(x, W_attn, b_attn, W_proj, b_proj)
    res = run_bass_kernel_spmd(nc, in_maps, core_ids=list(range(8)))
    out = np.zeros((B, S, D), dtype=np.float32)
    for c in range(8):
        b = c // 4
        out[b] += res.results[c]["out"]
    return out


# revision 22
# speedup vs baseline: 1.0003x; 1.0003x over previous
"""Causal self-attention on 8 NeuronCores.

Sharding: data-parallel over batch (B=2) x tensor-parallel over heads
(16 heads -> 4 groups of 4), Megatron-style. Core c handles batch c//4,
head-group c%4. Each core computes its QKV projection slice, 4 heads of
causal attention, and a partial output projection; the host sums the 4
partials per batch element.

Single pipelined loop over 512-query chunks (S=2048, D=1024, HD=64,
4 local heads, fp16 operands / fp32 accumulation):
  x is transposed + fp16-cast on the host, so no on-device transposes.
  Attention tiles are processed in 2-tile units sharing one PSUM score
  tile and one ACT exp call; the diagonal band is causally trimmed
  (widths 512/384/256/128) with 128-wide masks on Pool. The qkT/v
  projections of chunk j+1 and the output projection of chunk j-1 are
  interleaved as PE filler between attention heads of chunk j, keeping
  every engine busy. Softmax denominators ride a fused ones-row in the
  PV matmul; normalization = DVE reciprocal + Pool partition-broadcast
  + DVE multiply.
"""

import sys

sys.path.insert(0, "/opt/trn_rl_repo")

import numpy as np

import concourse.bass as bass
import concourse.bacc as bacc
import concourse.mybir as mybir
from concourse.bass import ts, ds
from concourse.tile import TileContext

B, S, D, H = 2, 2048, 1024, 16
HD = D // H  # 64
NH = 4  # heads per core
P = 128
DT = D // P  # 8 d-tiles
ST = S // P  # 16 s-tiles
NCHUNK = 4  # sq chunks of 512
CH = 512
F32 = mybir.dt.float32
F16 = mybir.dt.float16
LAU = 2  # score->PV unit lookahead

_nc_cache = None


def build_nc():
    nc = bacc.Bacc("TRN2", target_bir_lowering=False, debug=False, num_devices=8)

    xT_d = nc.dram_tensor("xT", [D, S], F16, kind="ExternalInput")
    wqk_d = nc.dram_tensor("wqk", [D, 2 * NH * HD], F16, kind="ExternalInput")
    wv_d = nc.dram_tensor("wv", [D, NH * HD], F16, kind="ExternalInput")
    wp_d = nc.dram_tensor("wp", [NH * HD, D], F16, kind="ExternalInput")
    bqk_d = nc.dram_tensor("bqk", [2 * NH * HD], F32, kind="ExternalInput")
    bv_d = nc.dram_tensor("bv", [NH * (HD + 1)], F32, kind="ExternalInput")
    bp_d = nc.dram_tensor("bp", [D], F32, kind="ExternalInput")
    out_d = nc.dram_tensor("out", [S, D], F32, kind="ExternalOutput")

    with TileContext(nc) as tc:
        import contextlib

        stack = contextlib.ExitStack()
        with stack:
            consts = stack.enter_context(tc.tile_pool(name="consts", bufs=1))
            xt_pool = stack.enter_context(tc.tile_pool(name="xt", bufs=2))
            yt_pool = stack.enter_context(tc.tile_pool(name="yt", bufs=4))
            pt_pool = stack.enter_context(tc.tile_pool(name="pt", bufs=6))
            ot_pool = stack.enter_context(tc.tile_pool(name="ot", bufs=3))
            rec_pool = stack.enter_context(tc.tile_pool(name="rec", bufs=3))
            mm_ps = stack.enter_context(tc.tile_pool(name="mm", bufs=2, space="PSUM"))
            sc_ps = stack.enter_context(tc.tile_pool(name="sc", bufs=2, space="PSUM"))
            pv_ps = stack.enter_context(tc.tile_pool(name="pv", bufs=2, space="PSUM"))

            # ---- weights / constants ----
            wqk_sb = consts.tile([P, DT, 2 * NH * HD], F16)
            wv_sb = consts.tile([P, DT, NH * HD], F16)
            wp_sb = consts.tile([P, 2, D], F16)
            bqk_sb = consts.tile([P, 4], F32)
            bv_sb = consts.tile([1, NH * (HD + 1)], F32)
            bv_rep = consts.tile([P, NH * (HD + 1)], F32)
            bp_sb = consts.tile([1, D], F32)
            bp_rep = consts.tile([P, D], F32)

            # persistent activations
            qkT_sb = consts.tile([P, 4, S], F16)  # q: tiles 0-1, k: tiles 2-3
            v_sb = consts.tile([P, ST, NH * (HD + 1)], F16)
            v4 = v_sb.rearrange("p st (h c) -> p st h c", c=HD + 1)
            bv4 = bv_rep.rearrange("p (h c) -> p h c", c=HD + 1)

            xT_ap = xT_d[:].rearrange("(dt p) s -> p dt s", p=P)
            wqk_ap = wqk_d[:].rearrange("(do p) f -> p do f", p=P)

            # ---- prologue DMAs, split + ordered for fast PE start ----
            nc.sync.dma_start(wqk_sb[:, :, :P], wqk_ap[:, :, :P])
            xts = {0: xt_pool.tile([P, DT, CH], F16, name="xt")}
            nc.sync.dma_start(xts[0][:, :4, :], xT_ap[:, :4, ds(0, CH)])
            nc.sync.dma_start(xts[0][:, 4:, :], xT_ap[:, 4:, ds(0, CH)])
            nc.sync.dma_start(bqk_sb, bqk_d[:].rearrange("(t p) -> p t", p=P))
            nc.sync.dma_start(
                wqk_sb[:, :, ts(2, P)], wqk_ap[:, :, ts(2, P)]
            )
            nc.sync.dma_start(wqk_sb[:, :, ts(1, P)], wqk_ap[:, :, ts(1, P)])
            nc.sync.dma_start(wqk_sb[:, :, ts(3, P)], wqk_ap[:, :, ts(3, P)])
            nc.sync.dma_start(wv_sb, wv_d[:].rearrange("(do p) f -> p do f", p=P))
            nc.sync.dma_start(bv_sb, bv_d[:].unsqueeze(0))
            nc.gpsimd.partition_broadcast(bv_rep, bv_sb)
            nc.sync.dma_start(wp_sb, wp_d[:].rearrange("(i p) f -> p i f", p=P))
            nc.sync.dma_start(bp_sb, bp_d[:].unsqueeze(0))
            nc.gpsimd.partition_broadcast(bp_rep, bp_sb)
            nc.gpsimd.memset(v4[:, :, :, HD : HD + 1], 1.0)

            # ---- filler unit emitters ----
            def emit_qk_ft(j, ft):
                xt = xts[j]
                pp = mm_ps.tile([P, CH], F32, name="pp", tag="mm")
                for do in range(DT):
                    nc.tensor.matmul(
                        pp,
                        wqk_sb[:, do, ts(ft, P)],
                        xt[:, do, :],
                        start=(do == 0),
                        stop=(do == DT - 1),
                    )
                nc.vector.tensor_scalar_add(
                    qkT_sb[:, ft, ds(j * CH, CH)], pp, bqk_sb[:, ft : ft + 1]
                )

            def emit_v_st(j, st4):
                xt = xts[j]
                st = 4 * j + st4
                pv = mm_ps.tile([P, CH], F32, name="pv", tag="mm")
                for do in range(DT):
                    nc.tensor.matmul(
                        pv[:, : NH * HD],
                        xt[:, do, ts(st4, P)],
                        wv_sb[:, do, :],
                        start=(do == 0),
                        stop=(do == DT - 1),
                    )
                nc.vector.tensor_tensor(
                    v4[:, st, :, :HD],
                    pv[:, : NH * HD].rearrange("p (h c) -> p h c", c=HD),
                    bv4[:, :, :HD],
                    mybir.AluOpType.add,
                )

            def emit_outproj_st(j, yt, st4, alt=False):
                o_t = ot_pool.tile([P, D], F32, name="o_t")
                for c in range(2):
                    if alt and c == 1:
                        op = sc_ps.tile([P, 2 * CH], F32, name="op2", tag="sc")[
                            :, :CH
                        ]
                    else:
                        op = mm_ps.tile([P, CH], F32, name="op", tag="mm")
                    nc.tensor.matmul(
                        op,
                        yt[:, 0, ts(st4, P)],
                        wp_sb[:, 0, ds(c * CH, CH)],
                        start=True,
                        stop=False,
                    )
                    nc.tensor.matmul(
                        op,
                        yt[:, 1, ts(st4, P)],
                        wp_sb[:, 1, ds(c * CH, CH)],
                        start=False,
                        stop=True,
                    )
                    nc.vector.tensor_tensor(
                        o_t[:, ds(c * CH, CH)],
                        op,
                        bp_rep[:, ds(c * CH, CH)],
                        mybir.AluOpType.add,
                    )
                nc.sync.dma_start(out_d[ts(4 * j + st4, P), :], o_t)

            def proj_units(j):
                return [lambda ft=ft: emit_qk_ft(j, ft) for ft in (0, 2, 1, 3)] + [
                    lambda s=s: emit_v_st(j, s) for s in range(4)
                ]

            # prologue: chunk-0 projections
            for u in proj_units(0):
                u()

            # ---- main loop ----
            # outproj of chunk jp is deferred into chunk DEFER[jp] as PE
            # filler (chunk 3 has no projection filler and is ACT-paced)
            DEFER = {0: 2, 1: 3, 2: 3}
            yts = {}
            for j in range(NCHUNK):
                filler = []
                if j + 1 < NCHUNK:
                    xts[j + 1] = xt_pool.tile([P, DT, CH], F16, name="xt")
                    nc.sync.dma_start(
                        xts[j + 1], xT_ap[:, :, ds((j + 1) * CH, CH)]
                    )
                for jp, jd in DEFER.items():
                    if jd == j:
                        yp = yts.pop(jp)
                        filler += [
                            lambda s=s, yp=yp, jp=jp: emit_outproj_st(jp, yp, s)
                            for s in range(4)
                        ]
                if j + 1 < NCHUNK:
                    filler += proj_units(j + 1)

                # fractional filler pacing across this chunk's attention units;
                # the last chunk reserves units to cover its tail norm chain
                tail_units = []
                if j == NCHUNK - 1:
                    tail_units = filler[-3:]
                    filler = filler[:-3]
                total_units = NH * (2 + 2 * j)
                frac = len(filler) / total_units
                acc = -2.0 * frac if j == 0 else 0.0  # chunk 0 waits on xt DMA

                def maybe_filler():
                    nonlocal acc
                    acc += frac
                    while acc >= 1.0 and filler:
                        filler.pop(0)()
                        acc -= 1.0

                yt = yt_pool.tile([P, 2, CH], F16, name="yt")
                yts[j] = yt
                pending = []  # (h, pvp, rec) queue awaiting broadcast+normalize

                def flush_norm():
                    if not pending:
                        return
                    hN, pvpN, recN = pending.pop(0)
                    recb = rec_pool.tile([HD, CH], F32, name="recb")
                    nc.gpsimd.partition_broadcast(recb, recN)
                    nc.vector.tensor_tensor(
                        yt[(hN % 2) * HD : (hN % 2) * HD + HD, hN // 2, :],
                        pvpN[0:HD, :],
                        recb,
                        mybir.AluOpType.mult,
                    )

                # units of <=2 tiles: (tile, width, q-offset, psum-offset)
                units = [
                    [(4 * j, CH, 0, 0), (4 * j + 1, 384, P, CH)],
                    [(4 * j + 2, 256, 2 * P, 0), (4 * j + 3, P, 3 * P, 256)],
                ] + [
                    [(d, CH, 0, 0), (d + 1, CH, 0, CH)]
                    for d in range(0, 4 * j, 2)
                ]
                nu = len(units)
                pvps = {}
                pts = {}

                def emit_scores(h, ui):
                    po = (h % 2) * HD
                    unit = units[ui]
                    sc = sc_ps.tile([P, 2 * CH], F32, name="sc", tag="sc")
                    for t, w, qoff, off in unit:
                        nc.tensor.matmul(
                            sc[:, ds(off, w)],
                            qkT_sb[po : po + HD, 2 + h // 2, ts(t, P)],
                            qkT_sb[po : po + HD, h // 2, ds(j * CH + qoff, w)],
                            start=True,
                            stop=True,
                        )
                    cov = unit[-1][3] + unit[-1][1]
                    pt = pt_pool.tile([P, 2 * CH], F16, name="pt")
                    nc.scalar.activation(
                        pt[:, :cov], sc[:, :cov],
                        mybir.ActivationFunctionType.Exp, scale=0.125,
                    )
                    if ui < 2:
                        for t, w, qoff, off in unit:
                            # mask upper triangle of the diagonal block
                            nc.gpsimd.affine_select(
                                out=pt[:, ds(off, P)],
                                in_=pt[:, ds(off, P)],
                                compare_op=mybir.AluOpType.is_ge,
                                fill=0.0,
                                base=0,
                                pattern=[[1, P]],
                                channel_multiplier=-1,
                            )
                    pts[(h, ui)] = pt

                def emit_pv(h, ui):
                    if ui == 0:
                        pvps[h] = pv_ps.tile([P, CH], F32, name="pvp", tag="pv")
                    pvp = pvps[h]
                    unit = units[ui]
                    pt = pts.pop((h, ui))
                    for k, (t, w, qoff, off) in enumerate(unit):
                        nc.tensor.matmul(
                            pvp[: HD + 1, ds(qoff, w)],
                            v_sb[:, t, ds(h * (HD + 1), HD + 1)],
                            pt[:, ds(off, w)],
                            start=(ui == 0 and k == 0),
                            stop=(ui == nu - 1 and k == len(unit) - 1),
                            skip_group_check=True,
                        )
                    if ui == nu - 1:
                        rec = rec_pool.tile([1, CH], F32, name="rec")
                        nc.vector.reciprocal(rec, pvp[HD : HD + 1, :])
                        pending.append((h, pvp, rec))

                stream = [(h, ui) for h in range(NH) for ui in range(nu)]
                for idx in range(len(stream) + LAU):
                    if idx < len(stream):
                        emit_scores(*stream[idx])
                    if idx >= LAU:
                        h, ui = stream[idx - LAU]
                        emit_pv(h, ui)
                        if ui == min(1, nu - 1) and h > 0:
                            flush_norm()  # normalize head h-1
                    maybe_filler()

                # tail: remaining filler covers the last norm chain latency
                while filler:
                    filler.pop(0)()
                for u in tail_units:
                    u()
                while pending:
                    flush_norm()

            # final chunk's output projection (alternate PSUM pools: the
            # attention pools are free by now, so rotation never stalls)
            yp = yts.pop(NCHUNK - 1)
            for st4 in range(4):
                emit_outproj_st(NCHUNK - 1, yp, st4, alt=True)

    nc.compile()
    return nc


def make_in_maps(x, W_attn, b_attn, W_proj, b_proj):
    x = np.asarray(x, dtype=np.float32)
    W_attn = np.asarray(W_attn, dtype=np.float32)
    b_attn = np.asarray(b_attn, dtype=np.float32)
    W_proj = np.asarray(W_proj, dtype=np.float32)
    b_proj = np.asarray(b_proj, dtype=np.float32)
    GF = NH * HD  # 256 features per group
    in_maps = []
    for c in range(8):
        b, g = divmod(c, 4)
        sl = slice(g * GF, (g + 1) * GF)
        wqk = np.concatenate(
            [W_attn[:, sl], W_attn[:, D + g * GF : D + (g + 1) * GF]], axis=1
        )
        bqk = np.concatenate([b_attn[sl], b_attn[D + g * GF : D + (g + 1) * GF]])
        wv = W_attn[:, 2 * D + g * GF : 2 * D + (g + 1) * GF]
        bv_flat = b_attn[2 * D + g * GF : 2 * D + (g + 1) * GF]
        bv = np.zeros(NH * (HD + 1), dtype=np.float32)
        for h in range(NH):
            bv[h * (HD + 1) : h * (HD + 1) + HD] = bv_flat[h * HD : (h + 1) * HD]
        in_maps.append(
            {
                "xT": np.ascontiguousarray(x[b].T.astype(np.float16)),
                "wqk": np.ascontiguousarray(wqk.astype(np.float16)),
                "wv": np.ascontiguousarray(wv.astype(np.float16)),
                "wp": np.ascontiguousarray(W_proj[sl, :].astype(np.float16)),
                "bqk": np.ascontiguousarray(bqk),
                "bv": bv,
                "bp": (b_proj if g == 0 else np.zeros_like(b_proj)).copy(),
            }
        )
    return in_maps


def kernel(x, W_attn, b_attn, W_proj, b_proj):
    global _nc_cache
    from concourse.bass_utils import run_bass_kernel_spmd

    if _nc_cache is None:
        _nc_cache = build_nc()
    nc = _nc_cache
    in_maps = make_in_maps(x, W_attn, b_attn, W_proj, b_proj)
    res = run_bass_kernel_spmd(nc, in_maps, core_ids=list(range(8)))
    out = np.zeros((B, S, D), dtype=np.float32)
    for c in range(8):
        b = c // 4
        out[b] += res.results[c]["out"]
    return out
